# revision 1
# baseline (speedup 1.0000x reference)
"""Trainium2 Bass kernel for nn_PluckettLuceKeibaBetting (v2).

B=8192 races x H=18 horses -> (8192, 6360) bet-type probabilities.
Pure data-parallel across 8 NeuronCores (1024 races each, 8 tiles of 128).

v2 over the f32 baseline:
  - bf16 output tile + bf16 DMA (halves the dominant HBM write traffic);
    host upcasts to f32.
  - umatan computed entirely on ScalarE: one f32r matmul gathers
    sf+ss-ln(d1-ef) per ordered pair, one Exp with bias=-ln(d1) emits
    umatan directly (kills the DVE stt and the separate GF gather).
  - all first-level gathers f32r (1 cyc/col on PE vs 4 for exact f32);
    stk1 transpose also f32r.
  - second-level tensors (uq/hpg/rq/w) held bf16 -> bf16 transposes and
    bf16 second-level gather matmuls; rq gather gains a 19th column that
    yields sq = sum(UQ) for free.
  - d1-SE subtractions moved to ScalarE (Identity with bias=d1).
  - wide in 4 ops: A=UQ-SRQ, B=(-SE)*A, C=umaren-SW2, wide=B+C (GPSIMD).
  - all per-tile Exp/Ln scalar work hoisted into a merged prelude over all
    8 race-tiles (one Exp->Ln->Exp activation-table sequence per kernel
    instead of per tile, and 8x fewer small DVE/ACT ops).
  - emission is software-pipelined: each iteration emits stage3(t-2)
    [sanrentan+DMA], stage1(t) [gathers+exps+recips], stage2(t-1)
    [transposes+rq/wide/sanrenpuku], so the in-order engines always have
    a stage of independent work in flight.
  - input DMAs issued before the ~31KB/partition of constants (G_T last)
    so the pipeline fill isn't blocked behind constant traffic.

Closed forms (d1 = sum e, g_j = 1/(d1-e_j), h_fs = 1/(d1-ef-es)):
  tansho[j]   = ej/d1
  umatan[f,s] = exp(sf+ss+ln(g_f)-ln(d1))
  umaren{a,b} = exp(sa+sb-ln(d1))*(ga+gb)
  q[f,s]      = umatan*h306;  UQ{a,b} = umaren*h153
  wide{a,b}   = umaren - (ea+eb)*(UQ-SRQ) - SW2,
                SRQ = rq_a+rq_b, SW2 = ea*rq_a+eb*rq_b, rq_j = sum UQ over
                pairs containing j
  fukusho     = tansho + P2nd (+P3rd if >7 running), P3 = e*(sq-rq)
  sanrenpuku  = exp(sa+sb+sc-ln(d1)) * BR, BR = 3-pair gather of h*(gx+gy)
  sanrentan   = q[opair] * e[third]  (ordered-pair-major, 16 thirds/pair)
"""

import itertools
import numpy as np

H = 18
B = 8192
NCORES = 8
BC = B // NCORES  # 1024 races per core
P = 128
NT = BC // P      # 8 race-tiles per core
N_PAIR = 153
N_OPAIR = 306
N_TRIP = 816
N_PERM = 4896
OUT_D = 6360

OFF_TANSHO = 0
OFF_FUKU = 18
OFF_UMAREN = 36
OFF_WIDE = 189
OFF_UMATAN = 342
OFF_SANPUKU = 648
OFF_SANTAN = 1464

ET_CHUNKS = [(i * 512, 512) for i in range(9)] + [(9 * 512, 288)]
# chunk indices whose q*ET multiply runs on GPSIMD (via an ACT PSUM->SBUF copy)
GP_CHUNKS = frozenset({4, 5, 6, 7, 8, 9})

# stk1 column / TRN row layout: S 0:18, LN(d1-e) 18:36, ES 36:54, G 54:72
R_S, R_LN, R_ES, R_G = 0, 18, 36, 54
TRNROWS = 72


def _build_consts():
    perms3 = np.array(list(itertools.permutations(range(H), 3)), dtype=np.int32)
    T3 = perms3[:, 2]
    opairs = list(itertools.permutations(range(H), 2))
    combos2 = list(itertools.combinations(range(H), 2))
    combos3 = list(itertools.combinations(range(H), 3))

    pair_id = {}
    for i, (a, b) in enumerate(combos2):
        pair_id[(a, b)] = i
        pair_id[(b, a)] = i

    M_2HOT = np.zeros((H, N_PAIR), np.float32)
    for j, (a, b) in enumerate(combos2):
        M_2HOT[a, j] = 1.0
        M_2HOT[b, j] = 1.0

    # ---- first-level f32r gathers (lhsT = trn[0:72]) ----
    # C_MA: SE306 = ef+es per ordered pair
    C_MA = np.zeros((TRNROWS, N_OPAIR), np.float32)
    for j, (f, s) in enumerate(opairs):
        C_MA[R_ES + f, j] += 1.0
        C_MA[R_ES + s, j] += 1.0
    # C_MB: [SG(153) | SE153(153)]
    C_MB = np.zeros((TRNROWS, 306), np.float32)
    C_MB[R_G:R_G + H, 0:153] = M_2HOT
    C_MB[R_ES:R_ES + H, 153:306] = M_2HOT
    # C_LQ: [LQ306 = sf+ss-ln(d1-ef) | SC2 = sa+sb | zero pad col]
    C_LQ = np.zeros((TRNROWS, 460), np.float32)
    for j, (f, s) in enumerate(opairs):
        C_LQ[R_S + f, j] += 1.0
        C_LQ[R_S + s, j] += 1.0
        C_LQ[R_LN + f, j] -= 1.0
    C_LQ[R_S:R_S + H, 306:459] = M_2HOT
    # C_L3: sa+sb+sc per triple (S rows only -> 18-row lhsT)
    C_L3 = np.zeros((H, N_TRIP), np.float32)
    for j, (a, b, c) in enumerate(combos3):
        C_L3[a, j] += 1.0
        C_L3[b, j] += 1.0
        C_L3[c, j] += 1.0
    # ET: e_third per permutation (ES rows)
    G_T = np.zeros((TRNROWS, N_PERM), np.float32)
    for j, t in enumerate(T3):
        G_T[R_ES + t, j] = 1.0

    consts_f32 = dict(
        C_MA=C_MA, C_MB=C_MB, C_LQ=C_LQ, C_L3=C_L3, G_T=G_T,
        IDENT=np.eye(128, dtype=np.float32),
    )

    # ---- second-level bf16 gathers (lhsT = bf16 transposes of stk2) ----
    # stk2 cols: uq 0:153 | hpg 153:306 | rq 306:324 | w 324:342
    # chunk A = cols 0:128 (uq 0:128), B = cols 128:256 (uq 128:153 rows
    # 0:25, hpg 0:103 rows 25:128), C = cols 256:342 (hpg 103:153 rows
    # 0:50, rq rows 50:68, w rows 68:86)
    M_RQ = np.zeros((N_PAIR, H), np.float32)
    for i, (a, b) in enumerate(combos2):
        M_RQ[i, a] = 1.0
        M_RQ[i, b] = 1.0
    M_BR = np.zeros((N_PAIR, N_TRIP), np.float32)
    for j, (a, b, c) in enumerate(combos3):
        M_BR[pair_id[(a, b)], j] += 1.0
        M_BR[pair_id[(a, c)], j] += 1.0
        M_BR[pair_id[(b, c)], j] += 1.0

    # rq gather + col 18 = sq = sum of UQ over all pairs
    C_RQ_A = np.zeros((128, 19), np.float32)
    C_RQ_A[:, 0:18] = M_RQ[0:128]
    C_RQ_A[:, 18] = 1.0
    C_RQ_B = np.zeros((128, 19), np.float32)
    C_RQ_B[0:25, 0:18] = M_RQ[128:153]
    C_RQ_B[0:25, 18] = 1.0

    C_BR_B = np.zeros((128, N_TRIP), np.float32)
    C_BR_B[25:128] = M_BR[0:103]
    C_BR_C = np.zeros((86, N_TRIP), np.float32)
    C_BR_C[0:50] = M_BR[103:153]
    C_RQW_C = np.zeros((86, N_OPAIR), np.float32)
    C_RQW_C[50:68, 0:153] = M_2HOT   # SRQ = rq_a+rq_b
    C_RQW_C[68:86, 153:306] = M_2HOT  # SW2 = w_a+w_b

    consts_bf16 = dict(
        C_RQ_A=C_RQ_A, C_RQ_B=C_RQ_B,
        C_BR_B=C_BR_B, C_BR_C=C_BR_C, C_RQW_C=C_RQW_C,
        IDENT_BF=np.eye(128, dtype=np.float32),
    )
    return consts_f32, consts_bf16


def _build_body(ctx, tc, out_ap, scores_ap, maskneg_ap, consts_f32, consts_bf16):
    import concourse.mybir as mybir
    from ml_dtypes import bfloat16

    nc = tc.nc
    f32 = mybir.dt.float32
    f32r = mybir.dt.float32r
    bf16 = mybir.dt.bfloat16
    Exp = mybir.ActivationFunctionType.Exp
    Log = mybir.ActivationFunctionType.Ln
    Identity = mybir.ActivationFunctionType.Identity
    MUL = mybir.AluOpType.mult
    SUB = mybir.AluOpType.subtract
    ADD = mybir.AluOpType.add

    def r(ap):
        return ap.bitcast(f32r)

    def mmr(out, lhsT, rhs, **kw):  # float32r full-rate matmul
        nc.tensor.matmul(out, r(lhsT), r(rhs), **kw)

    # ---- persistent constants (inputs DMA'd first; big late-use consts
    # like G_T last, so the prelude and first tiles aren't stuck behind
    # ~31KB/partition of constant traffic on the DMA queue) ----
    inpool = ctx.enter_context(tc.tile_pool(name="inp", bufs=1))
    mk = inpool.tile([P, NT], f32, tag="maskneg")
    nc.sync.dma_start(out=mk[:], in_=maskneg_ap.rearrange("(n p) o -> p (n o)", p=P))

    cpool = ctx.enter_context(tc.tile_pool(name="consts", bufs=1))
    C = {}
    order = ["IDENT", "C_MA", "C_MB", "C_LQ", "C_L3",
             "C_RQ_A", "C_RQ_B", "IDENT_BF", "C_RQW_C", "C_BR_B", "C_BR_C",
             "G_T"]

    def load_const(name):
        if name in consts_f32:
            arr = consts_f32[name]
            dram = nc.inline_tensor(arr, name=f"c_{name}")
            t = cpool.tile(list(arr.shape), f32, tag=f"c_{name}")
            nc.sync.dma_start(out=r(t[:]), in_=r(dram.ap()))
        else:
            arr16 = consts_bf16[name].astype(bfloat16)
            dram = nc.inline_tensor(arr16, name=f"c_{name}")
            t = cpool.tile(list(arr16.shape), bf16, tag=f"c_{name}")
            nc.sync.dma_start(out=t[:], in_=dram.ap())
        C[name] = t

    outp = ctx.enter_context(tc.tile_pool(name="out", bufs=4))
    wk = ctx.enter_context(tc.tile_pool(name="work", bufs=4))
    pps = ctx.enter_context(tc.tile_pool(name="ppsmall", bufs=3, space="PSUM"))
    ppt = ctx.enter_context(tc.tile_pool(name="pptrio", bufs=1, space="PSUM"))
    ppb = ctx.enter_context(tc.tile_pool(name="ppbig", bufs=1, space="PSUM"))
    ppe = ctx.enter_context(tc.tile_pool(name="ppet", bufs=2, space="PSUM"))

    # ================= pass 1: Exp/Ln prelude for all tiles =================
    # stk1_all[:, t, :] = [S | ln(d1-e) | ES | G] for race-tile t.  All ops
    # here are SBUF-only and span all NT tiles in one instruction, so the
    # Exp->Ln table switch happens once per kernel, not per tile.
    pre = ctx.enter_context(tc.tile_pool(name="prelude", bufs=1))
    stk1_all = pre.tile([P, NT, TRNROWS], f32, tag="stk1_all")
    S_all = stk1_all[:, :, R_S:R_S + 18]
    LND_all = stk1_all[:, :, R_LN:R_LN + 18]
    ES_all = stk1_all[:, :, R_ES:R_ES + 18]
    G_all = stk1_all[:, :, R_G:R_G + 18]
    nc.sync.dma_start(
        out=S_all, in_=scores_ap.rearrange("(t p) h -> p t h", p=P))
    for name in order:
        load_const(name)

    nc.scalar.activation(ES_all, S_all, Exp)
    d1_all = pre.tile([P, NT], f32, tag="d1_all")
    nc.vector.tensor_reduce(
        d1_all[:].unsqueeze(2), ES_all, axis=mybir.AxisListType.X, op=ADD)
    r1_all = pre.tile([P, NT], f32, tag="r1_all")
    nc.vector.reciprocal(r1_all[:], d1_all[:])

    def bcast(x):  # [P, NT] -> [P, NT, 18]
        return x.unsqueeze(2).broadcast_to([P, NT, 18])

    dmg_all = pre.tile([P, NT, 18], f32, tag="dmg_all")  # d1 - e
    nc.vector.tensor_sub(dmg_all[:], bcast(d1_all[:]), ES_all)
    nc.vector.reciprocal_approx_fast(out=G_all, in_=dmg_all[:])

    nlnd1_all = pre.tile([P, NT], f32, tag="nlnd1_all")
    nc.scalar.activation(nlnd1_all[:], r1_all[:], Log)
    nc.scalar.activation(LND_all, dmg_all[:], Log)

    # fuku2 = e/d1 + P2nd via z = e*g/d1, SS = sum_j z_j
    er1_all = pre.tile([P, NT, 18], f32, tag="er1_all")   # e/d1 = tansho
    nc.vector.tensor_mul(er1_all[:], ES_all, bcast(r1_all[:]))
    z_all = pre.tile([P, NT, 18], f32, tag="z_all")
    nc.vector.tensor_mul(z_all[:], er1_all[:], G_all)
    SS_all = pre.tile([P, NT], f32, tag="SS_all")
    nc.vector.tensor_reduce(
        SS_all[:].unsqueeze(2), z_all[:], axis=mybir.AxisListType.X, op=ADD)
    np2_all = pre.tile([P, NT, 18], f32, tag="np2_all")   # (z-SS)*e = -P2nd
    nc.vector.tensor_sub(np2_all[:], z_all[:], bcast(SS_all[:]))
    nc.vector.tensor_mul(np2_all[:], np2_all[:], ES_all)
    fuku2_all = pre.tile([P, NT, 18], f32, tag="fuku2_all")
    nc.vector.tensor_sub(fuku2_all[:], er1_all[:], np2_all[:])

    # ================= pass 2: skewed 3-stage pipeline (Exp set only) =======
    # Engines run their instruction streams strictly in order, so the long
    # per-tile cross-engine chain stalls every engine if tiles are emitted
    # whole.  Emitting S1(t), S2(t-1), S3(t-2) per iteration gives each
    # engine a full stage of independent work to hide the other stages'
    # dependency latency.
    TD = [dict() for _ in range(NT)]

    def stage1(t):
        d = TD[t]
        ot = outp.tile([P, OUT_D], bf16, tag="ot")
        d["ot"] = ot
        stk1 = stk1_all[:, t, :]
        d1 = d1_all[:, t:t + 1]
        nlnd1 = nlnd1_all[:, t:t + 1]

        # tansho
        nc.vector.tensor_copy(out=ot[:, 0:18], in_=er1_all[:, t, :])

        # transpose stk1 -> trn (rounded to f32r by the copy)
        ps_trn = pps.tile([TRNROWS, P], f32, tag="ps")
        nc.tensor.matmul(ps_trn[:], stk1, C["IDENT"][:], is_transpose=True)
        trn = wk.tile([TRNROWS, P], f32, tag="trn")
        d["trn"] = trn
        nc.scalar.copy(r(trn[:]), ps_trn[:])

        # first-level gathers
        ps_ma = pps.tile([P, N_OPAIR], f32, tag="ps")  # SE306
        mmr(ps_ma[:], trn[:], C["C_MA"][:], start=True, stop=True)
        ps_mb = pps.tile([P, 306], f32, tag="ps")      # [SG | SE153]
        mmr(ps_mb[:], trn[:], C["C_MB"][:], start=True, stop=True)
        ps_lq = pps.tile([P, 460], f32, tag="ps")      # [LQ306 | SC2 | pad]
        mmr(ps_lq[:], trn[:], C["C_LQ"][:], start=True, stop=True)
        SG = ps_mb[:, 0:153]
        SE153 = ps_mb[:, 153:306]

        # umatan = exp(LQ306 - ln d1) straight into the output tile
        nc.scalar.activation(
            ot[:, OFF_UMATAN:OFF_UMATAN + N_OPAIR], ps_lq[:, 0:306], Exp,
            bias=nlnd1)
        # umaren = exp(SC2 - ln d1) * SG
        esc2 = wk.tile([P, N_PAIR], bf16, tag="esc2")
        nc.scalar.activation(esc2[:], ps_lq[:, 306:459], Exp, bias=nlnd1)
        nc.vector.tensor_mul(
            ot[:, OFF_UMAREN:OFF_UMAREN + N_PAIR], esc2[:], SG)

        # h306 = 1/(d1 - SE306); q = umatan * h306
        h306p = wk.tile([P, N_OPAIR], f32, tag="h306p")
        nc.scalar.activation(h306p[:], ps_ma[:], Identity, bias=d1, scale=-1.0)
        h306 = wk.tile([P, N_OPAIR], f32, tag="h306")
        nc.vector.reciprocal_approx_fast(out=h306[:], in_=h306p[:])
        q = wk.tile([P, N_OPAIR], bf16, tag="q")
        d["q"] = q
        nc.vector.tensor_mul(q[:], ot[:, OFF_UMATAN:OFF_UMATAN + N_OPAIR],
                             h306[:])

        # h153 = 1/(d1 - SE153)
        d3p = wk.tile([P, N_PAIR], f32, tag="d3p")
        d["d3p"] = d3p
        nc.scalar.activation(d3p[:], SE153, Identity, bias=d1, scale=-1.0)
        h153 = wk.tile([P, N_PAIR], f32, tag="h153")
        nc.vector.reciprocal_approx_fast(out=h153[:], in_=d3p[:])

        # stk2 = [uq | hpg | rq | w] in bf16 for the second-level transposes
        stk2 = wk.tile([P, 342], bf16, tag="stk2")
        d["stk2"] = stk2
        nc.vector.tensor_mul(stk2[:, 0:153],
                             ot[:, OFF_UMAREN:OFF_UMAREN + N_PAIR], h153[:])
        nc.vector.tensor_mul(stk2[:, 153:306], h153[:], SG)

        # sanrenpuku first half: score-sum gather + exp (two <=512 chunks so
        # the PSUM tiles fit the small-pool rotation)
        eabc = wk.tile([P, N_TRIP], bf16, tag="eabc")
        d["eabc"] = eabc
        ps_l3a = pps.tile([P, 512], f32, tag="ps")
        mmr(ps_l3a[:], trn[R_S:R_S + 18], C["C_L3"][:, 0:512],
            start=True, stop=True)
        nc.scalar.activation(eabc[:, 0:512], ps_l3a[:], Exp, bias=nlnd1)
        ps_l3b = pps.tile([P, 304], f32, tag="ps")
        mmr(ps_l3b[:], trn[R_S:R_S + 18], C["C_L3"][:, 512:816],
            start=True, stop=True)
        nc.scalar.activation(eabc[:, 512:816], ps_l3b[:], Exp, bias=nlnd1)

    def stage2(t):
        d = TD[t]
        ot, stk2, d3p = d["ot"], d["stk2"], d["d3p"]
        ES = ES_all[:, t, :]
        d1 = d1_all[:, t:t + 1]
        mask_t = mk[:, t:t + 1]

        # transposes A,B (bf16 via PE) + rq/sq gather
        trio = ppt.tile([P, 3, P], bf16, tag="trio")
        ps_ta = trio[:, 0, :]
        nc.tensor.matmul(ps_ta, stk2[:, 0:128], C["IDENT_BF"][:],
                         is_transpose=True)
        ps_tb = trio[:, 1, :]
        nc.tensor.matmul(ps_tb, stk2[:, 128:256], C["IDENT_BF"][:],
                         is_transpose=True)
        ta = wk.tile([P, P], bf16, tag="ta")
        nc.scalar.copy(ta[:], ps_ta)
        tb = wk.tile([P, P], bf16, tag="tb")
        d["tb"] = tb
        nc.scalar.copy(tb[:], ps_tb)

        ps_rq = pps.tile([P, 19], f32, tag="ps")  # [rq | sq]
        nc.tensor.matmul(ps_rq[:], ta[:], C["C_RQ_A"][:], start=True,
                         stop=False)
        nc.tensor.matmul(ps_rq[:], tb[:], C["C_RQ_B"][:], start=False,
                         stop=True)

        # fukusho = fuku2 + mask*(rq - sq)*e   (mask = -1 when >7 running)
        np3 = wk.tile([P, 18], f32, tag="np3")
        nc.vector.scalar_tensor_tensor(
            np3[:], in0=ps_rq[:, 0:18], scalar=ps_rq[:, 18:19], in1=ES,
            op0=SUB, op1=MUL)
        nc.vector.scalar_tensor_tensor(
            ot[:, OFF_FUKU:OFF_FUKU + 18], in0=np3[:], scalar=mask_t,
            in1=fuku2_all[:, t, :], op0=MUL, op1=ADD)

        # rq/w into stk2, transpose chunk C, SRQW gather
        nc.scalar.copy(stk2[:, 306:324], ps_rq[:, 0:18])
        nc.vector.tensor_mul(stk2[:, 324:342], ES, ps_rq[:, 0:18])
        ps_tc = trio[0:86, 2, :]
        nc.tensor.matmul(ps_tc, stk2[:, 256:342], C["IDENT_BF"][:],
                         is_transpose=True)
        tc_t = wk.tile([86, P], bf16, tag="tc_t")
        d["tc_t"] = tc_t
        nc.scalar.copy(tc_t[:], ps_tc)
        ps_srqw = pps.tile([P, N_OPAIR], f32, tag="ps")  # [SRQ | SW2]
        nc.tensor.matmul(ps_srqw[:], tc_t[:], C["C_RQW_C"][:],
                         start=True, stop=True)

        # wide = umaren - SE*(UQ-SRQ) - SW2
        wA = wk.tile([P, N_PAIR], f32, tag="wA")
        nc.vector.tensor_sub(wA[:], stk2[:, 0:153], ps_srqw[:, 0:153])
        wB = wk.tile([P, N_PAIR], f32, tag="wB")  # (d3p-d1)*A = -SE*A
        nc.vector.scalar_tensor_tensor(
            wB[:], in0=d3p[:], scalar=d1, in1=wA[:], op0=SUB, op1=MUL)
        wC = wk.tile([P, N_PAIR], f32, tag="wC")
        nc.vector.tensor_sub(
            wC[:], ot[:, OFF_UMAREN:OFF_UMAREN + N_PAIR],
            ps_srqw[:, 153:306])
        nc.gpsimd.tensor_add(ot[:, OFF_WIDE:OFF_WIDE + N_PAIR], wB[:], wC[:])

        # sanrenpuku second half: 3-pair gather of hpg, then * eabc
        ps_br = ppb.tile([P, N_TRIP], f32, tag="big")
        for lo, hi in ((0, 512), (512, 816)):
            nc.tensor.matmul(ps_br[:, lo:hi], tb[:], C["C_BR_B"][:, lo:hi],
                             start=True, stop=False)
            nc.tensor.matmul(ps_br[:, lo:hi], tc_t[:], C["C_BR_C"][:, lo:hi],
                             start=False, stop=True)
        nc.vector.tensor_mul(
            ot[:, OFF_SANPUKU:OFF_SANPUKU + N_TRIP], d["eabc"][:], ps_br[:])

    def stage3(t):
        d = TD[t]
        ot, trn, q = d["ot"], d["trn"], d["q"]
        rows = slice(t * P, (t + 1) * P)

        # sanrentan: ET * q (broadcast over 16 thirds).  The out DMA is
        # split: the DVE-chunk half of the tile ships while GPSIMD still
        # works the back half, shortening the per-tile and drain tails.
        # last tile: nothing pipelines behind it, so shift chunks from the
        # slow GPSIMD path to DVE to shorten the drain
        gp_chunks = GP_CHUNKS if t < NT - 1 else frozenset({7, 8, 9})
        gp_list = sorted(gp_chunks)
        dve_list = [c for c in range(len(ET_CHUNKS)) if c not in gp_chunks]
        SPLIT = OFF_SANTAN + min(gp_chunks) * 512
        # GP chunks first: their ACT copies issue early so the slow GPSIMD
        # path starts sooner; its half of the tile ships as soon as done.
        for k, ci in enumerate(gp_list + dve_list):
            c0, w = ET_CHUNKS[ci]
            ps_et = ppe.tile([P, 512], f32, tag="et")
            mmr(ps_et[:, 0:w], trn[:], C["G_T"][:, c0:c0 + w],
                start=True, stop=True)
            npair = w // 16
            qb = q[:, c0 // 16: c0 // 16 + npair].unsqueeze(2).broadcast_to(
                [P, npair, 16])
            dst = ot[:, OFF_SANTAN + c0: OFF_SANTAN + c0 + w].rearrange(
                "p (a b) -> p a b", b=16)
            if ci in gp_chunks:
                et_sb = wk.tile([P, 512], bf16, tag="et_sb")
                nc.scalar.copy(et_sb[:, 0:w], ps_et[:, 0:w])
                src = et_sb[:, 0:w].rearrange("p (a b) -> p a b", b=16)
                nc.gpsimd.tensor_tensor(out=dst, in0=src, in1=qb, op=MUL)
            else:
                src = ps_et[:, 0:w].rearrange("p (a b) -> p a b", b=16)
                nc.vector.tensor_tensor(out=dst, in0=src, in1=qb, op=MUL)
            if k == len(gp_list) - 1:
                nc.sync.dma_start(out=out_ap[rows, SPLIT:OUT_D],
                                  in_=ot[:, SPLIT:OUT_D])

        nc.sync.dma_start(out=out_ap[rows, 0:SPLIT], in_=ot[:, 0:SPLIT])
        TD[t] = {}

    for t in range(NT + 2):
        if 0 <= t - 2 < NT:
            stage3(t - 2)
        if t < NT:
            stage1(t)
        if 0 <= t - 1 < NT:
            stage2(t - 1)


def _build_bass():
    from contextlib import ExitStack
    import concourse.bacc as bacc
    import concourse.mybir as mybir
    import concourse.tile as tile

    consts_f32, consts_bf16 = _build_consts()
    nc = bacc.Bacc("TRN2", target_bir_lowering=False, debug=False,
                   enable_asserts=False, num_devices=NCORES)
    f32 = mybir.dt.float32
    bf16 = mybir.dt.bfloat16
    scores = nc.dram_tensor("scores", (BC, H), f32, kind="ExternalInput").ap()
    maskneg = nc.dram_tensor("maskneg", (BC, 1), f32, kind="ExternalInput").ap()
    out = nc.dram_tensor("out", (BC, OUT_D), bf16, kind="ExternalOutput").ap()

    with tile.TileContext(nc) as tc:
        with ExitStack() as ctx:
            _build_body(ctx, tc, out, scores, maskneg, consts_f32, consts_bf16)
    nc.compile()
    return nc


_cached_nc = None


def _get_nc():
    global _cached_nc
    if _cached_nc is None:
        _cached_nc = _build_bass()
    return _cached_nc


def kernel(scores, num_horses_running, _trace=False, _tmpdir=None):
    from concourse.bass_utils import run_bass_kernel_spmd

    nc = _get_nc()
    scores = np.ascontiguousarray(np.asarray(scores), dtype=np.float32)
    nhr = np.asarray(num_horses_running)
    maskneg = np.where(nhr > 7, -1.0, 0.0).astype(np.float32).reshape(B, 1)

    in_maps = [
        {"scores": scores[c * BC:(c + 1) * BC],
         "maskneg": np.ascontiguousarray(maskneg[c * BC:(c + 1) * BC])}
        for c in range(NCORES)
    ]
    res = run_bass_kernel_spmd(nc, in_maps, core_ids=list(range(NCORES)),
                               trace=_trace, tmpdir=_tmpdir)
    out = np.concatenate(
        [np.asarray(r["out"]).astype(np.float32) for r in res.results], axis=0)
    if _trace:
        return out, res
    return out



# revision 7
# speedup vs baseline: 1.0495x; 1.0495x over previous
"""Trainium2 Bass kernel for nn_PluckettLuceKeibaBetting (v3).

B=8192 races x H=18 horses -> (8192, 6360) bet-type probabilities.
Pure data-parallel across 8 NeuronCores (1024 races each, 8 tiles of 128).

v3 over v2:
  - umatan/sanrenpuku/sanrentan shipped as float8_e4m3 scaled x128 in a
    second DRAM tensor (host upcasts and unscales); cuts the dominant
    output HBM write from 12720 to 6702 B/row.  These segments carry
    only ~2.3% of the output norm^2, so fp8's ~3% RMS quantization
    moves total rel err from 3.3e-3 to ~4e-3 (gate 2e-2).
  - single Identity over [SE306|SE153] and single reciprocal over 459
    cols (merged d36/h36); one C_MA' gather emits both SE blocks.
  - first-level gather lhsT slimmed to the rows actually used
    (C_LQ 36 rows, C_L3/G_T 18 rows) - 4x less constant DMA.
  - sanrentan in 1024-col chunks (5 per tile instead of 10): PSUM
    re-planned (l3/br through the small-pool rotation, ppe 2x2 banks).
  - engine rebalance: uq/wB/wide on GPSIMD(Pool), q' on DVE, Pool
    sanrentan chunks fed by ACT psum->bf16 copies.

Closed forms (d1 = sum e, g_j = 1/(d1-e_j), h = 1/(d1-ea-eb)):
  tansho[j]   = ej/d1
  umatan[f,s] = exp(sf+ss-ln(d1-ef)-ln d1)
  umaren{a,b} = exp(sa+sb-ln d1)*(ga+gb)
  q[f,s]      = umatan*h306;  UQ{a,b} = umaren*h153
  wide{a,b}   = umaren - (ea+eb)*(UQ-SRQ) - SW2
  fukusho     = tansho + P2nd (+P3rd if >7 running)
  sanrenpuku  = exp(sa+sb+sc-ln d1) * BR, BR = 3-pair gather of h*(gx+gy)
  sanrentan   = q[opair] * e[third]  (ordered-pair-major, 16 thirds/pair)
"""

import itertools
import math
import numpy as np

H = 18
B = 8192
NCORES = 8
BC = B // NCORES  # 1024 races per core
P = 128
NT = BC // P      # 8 race-tiles per core
N_PAIR = 153
N_OPAIR = 306
N_TRIP = 816
N_PERM = 4896
OUT_D = 6360

# bf16 block (otA): tansho | fukusho | umaren | wide
A_D = 342
OFFA_TANSHO = 0
OFFA_FUKU = 18
OFFA_UMAREN = 36
OFFA_WIDE = 189
# fp8 block (otB, x128): umatan | sanrenpuku | sanrentan
B_D = 6018
OFFB_UMATAN = 0
OFFB_SANPUKU = 306
OFFB_SANTAN = 1122
FP8_SCALE = 128.0
LN_SCALE = math.log(FP8_SCALE)

# sanrentan chunks (start, width) in perm space
ET_CHUNKS = [(0, 1024), (1024, 1024), (2048, 1024), (3072, 1024), (4096, 800)]
# chunk ids whose multiply runs on GPSIMD/Pool (fed by an ACT psum->bf16 copy)
GP_CHUNKS = frozenset({3, 4})
GP_CHUNKS_LAST = frozenset({4})

# stk1 column / trn row layout: S 0:18, LN(d1-e) 18:36, ES 36:54,
# pad 54:64, G 64:82.  PE lhsT slices must start at partition 0/32/64 and a
# base-32 slice spans at most 32 partitions, hence the padded layout.
R_S, R_LN, R_ES, R_G = 0, 18, 36, 64
TRNROWS = 82


def _build_consts():
    perms3 = np.array(list(itertools.permutations(range(H), 3)), dtype=np.int32)
    T3 = perms3[:, 2]
    opairs = list(itertools.permutations(range(H), 2))
    combos2 = list(itertools.combinations(range(H), 2))
    combos3 = list(itertools.combinations(range(H), 3))

    pair_id = {}
    for i, (a, b) in enumerate(combos2):
        pair_id[(a, b)] = i
        pair_id[(b, a)] = i

    M_2HOT = np.zeros((H, N_PAIR), np.float32)
    for j, (a, b) in enumerate(combos2):
        M_2HOT[a, j] = 1.0
        M_2HOT[b, j] = 1.0

    # ---- first-level f32r gathers ----
    # PE lhsT base partition must be 0/32/64, so ES/G-row gathers use
    # lhsT = trn[32:...] with zero-padded const rows (ES at row 4, G at 22).
    # C_MA (lhsT = trn[32:54]): [SE306 | SE153]
    C_MA = np.zeros((22, 460), np.float32)
    for j, (f, s) in enumerate(opairs):
        C_MA[4 + f, j] += 1.0
        C_MA[4 + s, j] += 1.0
    C_MA[4:22, 306:459] = M_2HOT
    # C_SG (lhsT = trn[64:82]): SG153 (+pad col for even fp32r width)
    C_SG = np.zeros((H, 154), np.float32)
    C_SG[:, 0:153] = M_2HOT
    # C_LQ (lhsT = trn[0:36]): [LQ306 = sf+ss-ln(d1-ef) | SC2 = sa+sb | pad]
    C_LQ = np.zeros((36, 460), np.float32)
    for j, (f, s) in enumerate(opairs):
        C_LQ[R_S + f, j] += 1.0
        C_LQ[R_S + s, j] += 1.0
        C_LQ[R_LN + f, j] -= 1.0
    C_LQ[R_S:R_S + H, 306:459] = M_2HOT
    # C_L3 (lhsT = trn[R_S:R_S+18]): sa+sb+sc per triple
    C_L3 = np.zeros((H, N_TRIP), np.float32)
    for j, (a, b, c) in enumerate(combos3):
        C_L3[a, j] += 1.0
        C_L3[b, j] += 1.0
        C_L3[c, j] += 1.0
    # G_T (lhsT = trn[32:54]): e_third per permutation
    G_T = np.zeros((22, N_PERM), np.float32)
    for j, t in enumerate(T3):
        G_T[4 + t, j] = 1.0

    consts_f32 = dict(
        C_MA=C_MA, C_SG=C_SG, C_LQ=C_LQ, C_L3=C_L3, G_T=G_T,
        IDENT=np.eye(128, dtype=np.float32),
    )

    # ---- second-level bf16 gathers (lhsT = bf16 transposes of stk2) ----
    # stk2 cols: uq 0:153 | hpg 153:306 | rq 306:324 | w 324:342
    M_RQ = np.zeros((N_PAIR, H), np.float32)
    for i, (a, b) in enumerate(combos2):
        M_RQ[i, a] = 1.0
        M_RQ[i, b] = 1.0
    M_BR = np.zeros((N_PAIR, N_TRIP), np.float32)
    for j, (a, b, c) in enumerate(combos3):
        M_BR[pair_id[(a, b)], j] += 1.0
        M_BR[pair_id[(a, c)], j] += 1.0
        M_BR[pair_id[(b, c)], j] += 1.0

    # rq gather + col 18 = sq = sum of UQ over all pairs
    C_RQ_A = np.zeros((128, 19), np.float32)
    C_RQ_A[:, 0:18] = M_RQ[0:128]
    C_RQ_A[:, 18] = 1.0
    C_RQ_B = np.zeros((128, 19), np.float32)
    C_RQ_B[0:25, 0:18] = M_RQ[128:153]
    C_RQ_B[0:25, 18] = 1.0

    C_BR_B = np.zeros((128, N_TRIP), np.float32)
    C_BR_B[25:128] = M_BR[0:103]
    C_BR_C = np.zeros((86, N_TRIP), np.float32)
    C_BR_C[0:50] = M_BR[103:153]
    C_RQW_C = np.zeros((86, N_OPAIR), np.float32)
    C_RQW_C[50:68, 0:153] = M_2HOT   # SRQ = rq_a+rq_b
    C_RQW_C[68:86, 153:306] = M_2HOT  # SW2 = w_a+w_b

    consts_bf16 = dict(
        C_RQ_A=C_RQ_A, C_RQ_B=C_RQ_B,
        C_BR_B=C_BR_B, C_BR_C=C_BR_C, C_RQW_C=C_RQW_C,
        IDENT_BF=np.eye(128, dtype=np.float32),
    )
    return consts_f32, consts_bf16


def _build_body(ctx, tc, outA_ap, outB_ap, scores_ap, maskneg_ap,
                consts_f32, consts_bf16):
    import concourse.mybir as mybir
    from ml_dtypes import bfloat16

    nc = tc.nc
    f32 = mybir.dt.float32
    f32r = mybir.dt.float32r
    bf16 = mybir.dt.bfloat16
    fp8 = mybir.dt.float8e4
    Exp = mybir.ActivationFunctionType.Exp
    Log = mybir.ActivationFunctionType.Ln
    Identity = mybir.ActivationFunctionType.Identity
    MUL = mybir.AluOpType.mult
    SUB = mybir.AluOpType.subtract
    ADD = mybir.AluOpType.add

    def r(ap):
        return ap.bitcast(f32r)

    def mmr(out, lhsT, rhs, **kw):  # float32r full-rate matmul
        nc.tensor.matmul(out, r(lhsT), r(rhs), **kw)

    # ---- persistent constants (inputs DMA'd first, G_T last) ----
    inpool = ctx.enter_context(tc.tile_pool(name="inp", bufs=1))
    mk = inpool.tile([P, NT], f32, tag="maskneg")
    nc.sync.dma_start(out=mk[:], in_=maskneg_ap.rearrange("(n p) o -> p (n o)", p=P))

    cpool = ctx.enter_context(tc.tile_pool(name="consts", bufs=1))
    C = {}
    order = ["IDENT", "C_MA", "C_SG", "C_LQ", "C_L3",
             "C_RQ_A", "C_RQ_B", "IDENT_BF", "C_RQW_C", "C_BR_B", "C_BR_C",
             "G_T"]

    PADDED = {"C_MA": 32, "C_SG": 64, "G_T": 32}

    def load_const(name):
        pad = PADDED.get(name, 0)
        if name in consts_f32:
            arr = consts_f32[name]
            dram = nc.inline_tensor(arr, name=f"c_{name}")
            shape = [pad + arr.shape[0]] + list(arr.shape[1:])
            t = cpool.tile(shape, f32, tag=f"c_{name}")
            nc.sync.dma_start(out=r(t[pad:, :]), in_=r(dram.ap()))
        else:
            arr16 = consts_bf16[name].astype(bfloat16)
            dram = nc.inline_tensor(arr16, name=f"c_{name}")
            t = cpool.tile(list(arr16.shape), bf16, tag=f"c_{name}")
            nc.sync.dma_start(out=t[:], in_=dram.ap())
        C[name] = t[pad:, :] if pad else t[:]

    outpA = ctx.enter_context(tc.tile_pool(name="outA", bufs=4))
    outpB = ctx.enter_context(tc.tile_pool(name="outB", bufs=4))
    wk = ctx.enter_context(tc.tile_pool(name="work", bufs=4))
    pps = ctx.enter_context(tc.tile_pool(name="ppsmall", bufs=3, space="PSUM"))
    ppt = ctx.enter_context(tc.tile_pool(name="pptrio", bufs=1, space="PSUM"))
    ppe = ctx.enter_context(tc.tile_pool(name="ppet", bufs=2, space="PSUM"))

    # ================= pass 1: Exp/Ln prelude for all tiles =================
    pre = ctx.enter_context(tc.tile_pool(name="prelude", bufs=1))
    stk1_all = pre.tile([P, NT, TRNROWS], f32, tag="stk1_all")
    S_all = stk1_all[:, :, R_S:R_S + 18]
    LND_all = stk1_all[:, :, R_LN:R_LN + 18]
    ES_all = stk1_all[:, :, R_ES:R_ES + 18]
    G_all = stk1_all[:, :, R_G:R_G + 18]
    nc.sync.dma_start(
        out=S_all, in_=scores_ap.rearrange("(t p) h -> p t h", p=P))
    nc.vector.memset(stk1_all[:, :, 54:64], 0.0)
    for name in order:
        load_const(name)

    nc.scalar.activation(ES_all, S_all, Exp)
    d1_all = pre.tile([P, NT], f32, tag="d1_all")
    nc.vector.tensor_reduce(
        d1_all[:].unsqueeze(2), ES_all, axis=mybir.AxisListType.X, op=ADD)
    r1_all = pre.tile([P, NT], f32, tag="r1_all")
    nc.vector.reciprocal(r1_all[:], d1_all[:])

    def bcast(x):  # [P, NT] -> [P, NT, 18]
        return x.unsqueeze(2).broadcast_to([P, NT, 18])

    dmg_all = pre.tile([P, NT, 18], f32, tag="dmg_all")  # d1 - e
    nc.vector.tensor_sub(dmg_all[:], bcast(d1_all[:]), ES_all)
    nc.vector.reciprocal_approx_fast(out=G_all, in_=dmg_all[:])

    nlnd1_all = pre.tile([P, NT], f32, tag="nlnd1_all")
    nc.scalar.activation(nlnd1_all[:], r1_all[:], Log)
    nc.scalar.activation(LND_all, dmg_all[:], Log)
    # -ln d1 + ln 128 for the fp8-scaled exponentials
    nlnd1s_all = pre.tile([P, NT], f32, tag="nlnd1s_all")
    nc.vector.tensor_scalar_add(nlnd1s_all[:], nlnd1_all[:], LN_SCALE)

    # fuku2 = e/d1 + P2nd via z = e*g/d1, SS = sum_j z_j
    er1_all = pre.tile([P, NT, 18], f32, tag="er1_all")   # e/d1 = tansho
    nc.vector.tensor_mul(er1_all[:], ES_all, bcast(r1_all[:]))
    z_all = pre.tile([P, NT, 18], f32, tag="z_all")
    nc.vector.tensor_mul(z_all[:], er1_all[:], G_all)
    SS_all = pre.tile([P, NT], f32, tag="SS_all")
    nc.vector.tensor_reduce(
        SS_all[:].unsqueeze(2), z_all[:], axis=mybir.AxisListType.X, op=ADD)
    np2_all = pre.tile([P, NT, 18], f32, tag="np2_all")   # (z-SS)*e = -P2nd
    nc.vector.tensor_sub(np2_all[:], z_all[:], bcast(SS_all[:]))
    nc.vector.tensor_mul(np2_all[:], np2_all[:], ES_all)
    fuku2_all = pre.tile([P, NT, 18], f32, tag="fuku2_all")
    nc.vector.tensor_sub(fuku2_all[:], er1_all[:], np2_all[:])

    # ================= pass 2: skewed 3-stage pipeline ======================
    TD = [dict() for _ in range(NT)]

    def stage1(t):
        d = TD[t]
        otA = outpA.tile([P, A_D], bf16, tag="otA")
        otB = outpB.tile([P, B_D], fp8, tag="otB")
        d["otA"], d["otB"] = otA, otB
        stk1 = stk1_all[:, t, :]
        d1 = d1_all[:, t:t + 1]
        nlnd1 = nlnd1_all[:, t:t + 1]
        nlnd1s = nlnd1s_all[:, t:t + 1]

        # tansho
        nc.vector.tensor_copy(out=otA[:, 0:18], in_=er1_all[:, t, :])

        # transpose stk1 -> trn (rounded to f32r by the copy)
        ps_trn = pps.tile([TRNROWS, P], f32, tag="ps")
        nc.tensor.matmul(ps_trn[:], stk1, C["IDENT"], is_transpose=True)
        trn = wk.tile([TRNROWS, P], f32, tag="trn")
        d["trn"] = trn
        nc.scalar.copy(r(trn[:]), ps_trn[:])

        # first-level gathers
        ps_ma = pps.tile([P, 460], f32, tag="ps")   # [SE306 | SE153 | pad]
        mmr(ps_ma[:], trn[32:54], C["C_MA"], start=True, stop=True)
        ps_sg_t = pps.tile([P, 154], f32, tag="ps")  # [SG | pad]
        mmr(ps_sg_t[:], trn[64:82], C["C_SG"], start=True, stop=True)
        ps_sg = ps_sg_t[:, 0:153]
        ps_lq = pps.tile([P, 460], f32, tag="ps")   # [LQ306 | SC2 | pad]
        mmr(ps_lq[:], trn[0:36], C["C_LQ"], start=True, stop=True)

        # umatan' = 128*exp(LQ306 - ln d1) straight into the fp8 tile
        nc.scalar.activation(
            otB[:, OFFB_UMATAN:OFFB_UMATAN + N_OPAIR], ps_lq[:, 0:306], Exp,
            bias=nlnd1s)
        # umaren = exp(SC2 - ln d1) * SG (unscaled, bf16)
        esc2 = wk.tile([P, N_PAIR], bf16, tag="esc2")
        nc.scalar.activation(esc2[:], ps_lq[:, 306:459], Exp, bias=nlnd1)
        nc.vector.tensor_mul(
            otA[:, OFFA_UMAREN:OFFA_UMAREN + N_PAIR], esc2[:], ps_sg)

        # d36 = d1 - [SE306|SE153]; h36 = 1/d36
        d36 = wk.tile([P, 459], f32, tag="d36")
        d["d36"] = d36
        nc.scalar.activation(d36[:], ps_ma[:, 0:459], Identity, bias=d1, scale=-1.0)
        h36 = wk.tile([P, 459], f32, tag="h36")
        d["h36"] = h36
        nc.vector.reciprocal_approx_fast(out=h36[:], in_=d36[:])

        # q' = umatan' * h306 (bf16, carries the x128 scale)
        q = wk.tile([P, N_OPAIR], bf16, tag="q")
        d["q"] = q
        nc.vector.tensor_mul(q[:], otB[:, OFFB_UMATAN:OFFB_UMATAN + N_OPAIR],
                             h36[:, 0:306])

        # stk2 = [uq | hpg | rq | w] in bf16 for the second-level transposes
        stk2 = wk.tile([P, 342], bf16, tag="stk2")
        d["stk2"] = stk2
        nc.gpsimd.tensor_mul(stk2[:, 0:153],
                             otA[:, OFFA_UMAREN:OFFA_UMAREN + N_PAIR],
                             h36[:, 306:459])
        nc.vector.tensor_mul(stk2[:, 153:306], h36[:, 306:459], ps_sg)

        # sanrenpuku first half: score-sum gather + exp (x128 scale)
        eabc = wk.tile([P, N_TRIP], bf16, tag="eabc")
        d["eabc"] = eabc
        ps_l3a = pps.tile([P, 512], f32, tag="ps")
        mmr(ps_l3a[:], trn[R_S:R_S + 18], C["C_L3"][:, 0:512],
            start=True, stop=True)
        nc.scalar.activation(eabc[:, 0:512], ps_l3a[:], Exp, bias=nlnd1s)
        ps_l3b = pps.tile([P, 304], f32, tag="ps")
        mmr(ps_l3b[:], trn[R_S:R_S + 18], C["C_L3"][:, 512:816],
            start=True, stop=True)
        nc.scalar.activation(eabc[:, 512:816], ps_l3b[:], Exp, bias=nlnd1s)

    def stage2(t):
        d = TD[t]
        otA, otB, stk2, d36 = d["otA"], d["otB"], d["stk2"], d["d36"]
        ES = ES_all[:, t, :]
        d1 = d1_all[:, t:t + 1]
        mask_t = mk[:, t:t + 1]

        # transposes A,B (bf16 via PE) + rq/sq gather
        trio = ppt.tile([P, 3, P], bf16, tag="trio")
        ps_ta = trio[:, 0, :]
        nc.tensor.matmul(ps_ta, stk2[:, 0:128], C["IDENT_BF"],
                         is_transpose=True)
        ps_tb = trio[:, 1, :]
        nc.tensor.matmul(ps_tb, stk2[:, 128:256], C["IDENT_BF"],
                         is_transpose=True)
        tab = wk.tile([P, 2, P], bf16, tag="tab")
        d["tab"] = tab
        nc.scalar.copy(tab[:], trio[:, 0:2, :])
        ta = tab[:, 0, :]
        tb = tab[:, 1, :]

        ps_rq = pps.tile([P, 19], f32, tag="ps")  # [rq | sq]
        nc.tensor.matmul(ps_rq[:], ta, C["C_RQ_A"], start=True,
                         stop=False)
        nc.tensor.matmul(ps_rq[:], tb, C["C_RQ_B"], start=False,
                         stop=True)

        # fukusho = fuku2 + mask*(rq - sq)*e   (mask = -1 when >7 running)
        np3 = wk.tile([P, 18], f32, tag="np3")
        nc.vector.scalar_tensor_tensor(
            np3[:], in0=ps_rq[:, 0:18], scalar=ps_rq[:, 18:19], in1=ES,
            op0=SUB, op1=MUL)
        nc.vector.scalar_tensor_tensor(
            otA[:, OFFA_FUKU:OFFA_FUKU + 18], in0=np3[:], scalar=mask_t,
            in1=fuku2_all[:, t, :], op0=MUL, op1=ADD)

        # rq/w into stk2, transpose chunk C, SRQW gather
        nc.scalar.copy(stk2[:, 306:324], ps_rq[:, 0:18])
        nc.vector.tensor_mul(stk2[:, 324:342], ES, ps_rq[:, 0:18])
        ps_tc = trio[0:86, 2, :]
        nc.tensor.matmul(ps_tc, stk2[:, 256:342], C["IDENT_BF"],
                         is_transpose=True)
        tc_t = wk.tile([86, P], bf16, tag="tc_t")
        d["tc_t"] = tc_t
        nc.scalar.copy(tc_t[:], ps_tc)
        ps_srqw = pps.tile([P, N_OPAIR], f32, tag="ps")  # [SRQ | SW2]
        nc.tensor.matmul(ps_srqw[:], tc_t[:], C["C_RQW_C"],
                         start=True, stop=True)

        # wide = umaren - SE*(UQ-SRQ) - SW2
        wA = wk.tile([P, N_PAIR], f32, tag="wA")
        nc.vector.tensor_sub(wA[:], stk2[:, 0:153], ps_srqw[:, 0:153])
        wB = wk.tile([P, N_PAIR], f32, tag="wB")  # (d3p-d1)*A = -SE*A
        nc.vector.scalar_tensor_tensor(
            wB[:], in0=d36[:, 306:459], scalar=d1, in1=wA[:],
            op0=SUB, op1=MUL)
        wC = wk.tile([P, N_PAIR], f32, tag="wC")
        nc.vector.tensor_sub(
            wC[:], otA[:, OFFA_UMAREN:OFFA_UMAREN + N_PAIR],
            ps_srqw[:, 153:306])
        nc.gpsimd.tensor_add(otA[:, OFFA_WIDE:OFFA_WIDE + N_PAIR],
                             wB[:], wC[:])

        # sanrenpuku second half: 3-pair gather of hpg, then * eabc (fp8 out)
        for lo, hi in ((0, 512), (512, 816)):
            ps_br = pps.tile([P, hi - lo], f32, tag="ps")
            nc.tensor.matmul(ps_br[:], tb, C["C_BR_B"][:, lo:hi],
                             start=True, stop=False)
            nc.tensor.matmul(ps_br[:], tc_t[:], C["C_BR_C"][:, lo:hi],
                             start=False, stop=True)
            nc.vector.tensor_mul(
                otB[:, OFFB_SANPUKU + lo:OFFB_SANPUKU + hi],
                d["eabc"][:, lo:hi], ps_br[:])

    def stage3(t):
        d = TD[t]
        otA, otB, trn, q = d["otA"], d["otB"], d["trn"], d["q"]
        rows = slice(t * P, (t + 1) * P)

        gp_chunks = GP_CHUNKS if t < NT - 1 else GP_CHUNKS_LAST
        gp_list = sorted(gp_chunks)
        dve_list = [c for c in range(len(ET_CHUNKS)) if c not in gp_chunks]
        SPLIT = OFFB_SANTAN + ET_CHUNKS[min(gp_chunks)][0]
        # GP chunks first: their ACT copies issue early so the slow GPSIMD
        # path starts sooner; its half of the tile ships as soon as done.
        for k, ci in enumerate(gp_list + dve_list):
            c0, w = ET_CHUNKS[ci]
            ps_et = ppe.tile([P, 1024], f32, tag="et")
            for mo in range(0, w, 512):
                mw = min(512, w - mo)
                mmr(ps_et[:, mo:mo + mw],
                    trn[32:54], C["G_T"][:, c0 + mo:c0 + mo + mw],
                    start=True, stop=True)
            npair = w // 16
            qb = q[:, c0 // 16: c0 // 16 + npair].unsqueeze(2).broadcast_to(
                [P, npair, 16])
            dst = otB[:, OFFB_SANTAN + c0: OFFB_SANTAN + c0 + w].rearrange(
                "p (a b) -> p a b", b=16)
            if ci in gp_chunks:
                et_sb = wk.tile([P, 1024], bf16, tag="et_sb")
                nc.scalar.copy(et_sb[:, 0:w], ps_et[:, 0:w])
                src = et_sb[:, 0:w].rearrange("p (a b) -> p a b", b=16)
                nc.gpsimd.tensor_tensor(out=dst, in0=src, in1=qb, op=MUL)
            else:
                src = ps_et[:, 0:w].rearrange("p (a b) -> p a b", b=16)
                nc.vector.tensor_tensor(out=dst, in0=src, in1=qb, op=MUL)
            if k == len(gp_list) - 1:
                nc.sync.dma_start(out=outB_ap[rows, SPLIT:B_D],
                                  in_=otB[:, SPLIT:B_D])

        nc.sync.dma_start(out=outB_ap[rows, 0:SPLIT], in_=otB[:, 0:SPLIT])
        nc.sync.dma_start(out=outA_ap[rows, :], in_=otA[:])
        TD[t] = {}

    for t in range(NT + 2):
        if 0 <= t - 2 < NT:
            stage3(t - 2)
        if t < NT:
            stage1(t)
        if 0 <= t - 1 < NT:
            stage2(t - 1)


def _build_bass():
    from contextlib import ExitStack
    import concourse.bacc as bacc
    import concourse.mybir as mybir
    import concourse.tile as tile

    consts_f32, consts_bf16 = _build_consts()
    nc = bacc.Bacc("TRN2", target_bir_lowering=False, debug=False,
                   enable_asserts=False, num_devices=NCORES)
    f32 = mybir.dt.float32
    bf16 = mybir.dt.bfloat16
    fp8 = mybir.dt.float8e4
    scores = nc.dram_tensor("scores", (BC, H), f32, kind="ExternalInput").ap()
    maskneg = nc.dram_tensor("maskneg", (BC, 1), f32, kind="ExternalInput").ap()
    outA = nc.dram_tensor("outA", (BC, A_D), bf16, kind="ExternalOutput").ap()
    outB = nc.dram_tensor("outB", (BC, B_D), fp8, kind="ExternalOutput").ap()

    with tile.TileContext(nc) as tc:
        with ExitStack() as ctx:
            _build_body(ctx, tc, outA, outB, scores, maskneg,
                        consts_f32, consts_bf16)
    nc.compile()
    return nc


_cached_nc = None


def _get_nc():
    global _cached_nc
    if _cached_nc is None:
        _cached_nc = _build_bass()
    return _cached_nc


def kernel(scores, num_horses_running, _trace=False, _tmpdir=None):
    from concourse.bass_utils import run_bass_kernel_spmd

    nc = _get_nc()
    scores = np.ascontiguousarray(np.asarray(scores), dtype=np.float32)
    nhr = np.asarray(num_horses_running)
    maskneg = np.where(nhr > 7, -1.0, 0.0).astype(np.float32).reshape(B, 1)

    in_maps = [
        {"scores": scores[c * BC:(c + 1) * BC],
         "maskneg": np.ascontiguousarray(maskneg[c * BC:(c + 1) * BC])}
        for c in range(NCORES)
    ]
    res = run_bass_kernel_spmd(nc, in_maps, core_ids=list(range(NCORES)),
                               trace=_trace, tmpdir=_tmpdir)
    outs = []
    for rr in res.results:
        a = np.asarray(rr["outA"]).astype(np.float32)
        b = np.asarray(rr["outB"]).astype(np.float32) / FP8_SCALE
        outs.append(np.concatenate([a, b], axis=1))
    out = np.concatenate(outs, axis=0)
    if _trace:
        return out, res
    return out
